# revision 5
# baseline (speedup 1.0000x reference)
"""Dense 2-layer 2-head GAT for Trainium2 (Bass/Tile), data-parallel over batch.

v4 = v3 (fp8 DoubleRow scores, rank-1 row-normalized score gen, XBAR DMA
transposes, 3-op ELU) + software-pipelined layer boundaries:

  - DoubleRow pairs are adjacent chunks (2t, 2t+1), so pairs 0-3 of a half
    depend only on node chunks 0-7 (h slabs 0-1) and pairs 4-7 only on
    chunks 8-15. Layer prep is split into early/late pieces along that
    boundary, with per-piece psr tiles so nothing persists across pieces.
  - The next layer's early prep (h slabs 0-1, abrd, B'/b' chunks 0-7, h
    transposes 0-7) is emitted inside the current layer's last-half score
    loop, right after the deferred epilogue of the first half - ACT/PE/DMA
    execute it while DVE/Pool grind the remaining score tiles.
  - The current layer's last epilogue is deferred into the next layer's
    first-half loop (3 pairs in); the late prep pieces follow it.
"""

import os
from contextlib import ExitStack

import numpy as np

import concourse.bass as bass
import concourse.mybir as mybir
import concourse.tile as tile
from concourse.alu_op_type import AluOpType

F32 = mybir.dt.float32
F32R = mybir.dt.float32r
BF16 = mybir.dt.bfloat16
FP8 = mybir.dt.float8e4
AF = mybir.ActivationFunctionType

N = 2048
F = 256
D = 128
P = 128
ALPHA = 0.2
CSH = 3.0          # row-normalized exp shift
N_CORES = 8
DR = mybir.MatmulPerfMode.DoubleRow

# score-gen path per chunk slot within a (half, head): D=DVE-direct fp8 (2x),
# P=Pool-direct fp8 (1x)
PATHS = ["D", "P", "D", "P", "D", "D", "P", "D",
         "P", "D", "D", "P", "D", "P", "D", "P"]


def build_nc(n=N):
    from concourse import bacc
    nc = bacc.Bacc("TRN2", target_bir_lowering=False, debug=False,
                   enable_asserts=False, num_devices=N_CORES)

    x_d = nc.declare_dram_parameter("x", [n, F], F32, isOutput=False)
    W_d, b_d, a_d = {}, {}, {}
    for l in (0, 1):
        for h in (0, 1):
            W_d[l, h] = nc.declare_dram_parameter(f"W_{l}_{h}", [F, D], F32, isOutput=False)
            b_d[l, h] = nc.declare_dram_parameter(f"b_{l}_{h}", [D], F32, isOutput=False)
            a_d[l, h] = nc.declare_dram_parameter(f"a_{l}_{h}", [2 * D, 1], F32, isOutput=False)
    out_d = nc.declare_dram_parameter("out", [n, F], F32, isOutput=True)

    NJ = n // P          # node chunks of 128 (partition dim of score tiles)
    NPAIR = NJ // 2      # DoubleRow pairs (2t, 2t+1)
    IB = min(512, n)     # i-block width (one PSUM bank)
    NI = n // IB
    HW = 2 * IB          # half width (score tile free dim)
    NHALF = n // HW
    KH = HW // IB

    with tile.TileContext(nc) as tc, ExitStack() as ctx:
        const = ctx.enter_context(tc.tile_pool(name="const", bufs=1))
        persist = ctx.enter_context(tc.tile_pool(name="persist", bufs=1))
        headp = ctx.enter_context(tc.tile_pool(name="headp", bufs=4))
        up = ctx.enter_context(tc.tile_pool(name="up", bufs=10))
        hsbp = ctx.enter_context(tc.tile_pool(name="hsbp", bufs=6))
        epp = ctx.enter_context(tc.tile_pool(name="epp", bufs=2))
        smallp = ctx.enter_context(tc.tile_pool(name="smallp", bufs=6))
        ps_prep = ctx.enter_context(tc.tile_pool(name="ps_prep", bufs=2, space="PSUM"))
        ps_main = ctx.enter_context(tc.tile_pool(name="ps_main", bufs=2, space="PSUM"))
        ps_z = ctx.enter_context(tc.tile_pool(name="ps_z", bufs=1, space="PSUM"))

        # ---- constants ----
        negc2 = const.tile([P, 1], F32, tag="negc2", name="negc2")
        nc.vector.memset(negc2[:], -CSH / 2)
        negc = const.tile([P, 1], F32, tag="negc", name="negc")
        nc.vector.memset(negc[:], -CSH)
        # zp stationary: [P, 2, 16] fp8 (outer dual-row stride must be
        # 16B-aligned per s3_lw_dual_fp8_restrictions). Head hi's ones sit in
        # cols 2hi:2hi+2; cols 4-15 of head 0 are dummy ones so the unused
        # recip rows stay finite.
        ones2 = []
        for hi in range(2):
            o2 = const.tile([P, 2, 16], FP8, tag=f"o2{hi}", name=f"o2{hi}")
            nc.vector.memset(o2[:], 1.0 if hi == 0 else 0.0)
            if hi == 0:
                nc.vector.memset(o2[:, :, 2:4], 0.0)
            else:
                nc.vector.memset(o2[:, :, 2:4], 1.0)
            ones2.append(o2)
        ZR = 16
        sel4 = []
        for hi in range(2):
            s4f = const.tile([ZR, P], F32, tag=f"s4f{hi}", name=f"s4f{hi}")
            nc.gpsimd.memset(s4f[:], 0.0)
            nc.gpsimd.affine_select(
                out=s4f[:], in_=s4f[:], compare_op=AluOpType.not_equal,
                fill=1.0, base=-2 * hi, pattern=[[0, P]], channel_multiplier=1)
            s4 = const.tile([ZR, P], BF16, tag=f"s4{hi}", name=f"s4{hi}")
            nc.vector.tensor_copy(s4[:], s4f[:])
            sel4.append(s4)

        # ---- parameters ----
        Wt, bt, Alt, art = {}, {}, {}, {}
        for l in (0, 1):
            for h in (0, 1):
                Wt[l, h] = []
                for c in range(2):
                    wf = smallp.tile([P, D], F32, tag="wload", name="wload")
                    nc.sync.dma_start(out=wf[:], in_=W_d[l, h][c * P:(c + 1) * P, :])
                    w = const.tile([P, D], BF16, tag=f"W{l}{h}{c}", name=f"W{l}{h}{c}")
                    nc.vector.tensor_copy(w[:], wf[:])
                    Wt[l, h].append(w)
                b = const.tile([P, 1], F32, tag=f"b{l}{h}", name=f"b{l}{h}")
                nc.sync.dma_start(
                    out=b[:], in_=b_d[l, h][:].rearrange("(p o) -> p o", o=1))
                bt[l, h] = b
                alf = smallp.tile([P, 1], F32, tag="alload", name="alload")
                nc.sync.dma_start(out=alf[:], in_=a_d[l, h][0:P, 0:1])
                Al = const.tile([P, P], BF16, tag=f"Al{l}{h}", name=f"Al{l}{h}")
                nc.vector.tensor_copy(Al[:], alf[:].to_broadcast([P, P]))
                Alt[l, h] = Al
                arf = smallp.tile([P, 1], F32, tag="arload", name="arload")
                nc.sync.dma_start(out=arf[:], in_=a_d[l, h][P:2 * P, 0:1])
                ar2 = const.tile([P, 2], BF16, tag=f"ar{l}{h}", name=f"ar{l}{h}")
                nc.vector.tensor_copy(ar2[:], arf[:].to_broadcast([P, 2]))
                art[l, h] = ar2

        # ---- load x, convert to bf16, XBAR-transpose to xT [2 x (P, n)] ----
        xT = [persist.tile([P, n], BF16, tag=f"xT{f}", name=f"xT{f}") for f in range(2)]
        for c in range(NJ):
            xc = smallp.tile([P, F], F32, tag="xload", name="xload")
            nc.sync.dma_start(out=xc[:], in_=x_d[c * P:(c + 1) * P, :])
            xb = smallp.tile([P, F], BF16, tag="xb", name="xb")
            if c % 2 == 0:
                nc.vector.tensor_copy(xb[:], xc[:])
            else:
                nc.gpsimd.tensor_copy(xb[:], xc[:])
            for f in range(2):
                nc.sync.dma_start_transpose(
                    out=xT[f][:, c * P:(c + 1) * P], in_=xb[:, f * P:(f + 1) * P])

        X1T = [persist.tile([P, n], BF16, tag=f"X1T{f}", name=f"X1T{f}") for f in range(2)]
        X2T = [persist.tile([P, n], BF16, tag=f"X2T{f}", name=f"X2T{f}") for f in range(2)]

        def mk_state(li):
            hd = [dict(), dict()]
            for hi in range(2):
                hd[hi]["hT"] = headp.tile([P, n], BF16, tag="hT", name=f"hT{li}{hi}")
                hd[hi]["abrd"] = headp.tile([P, n], BF16, tag="abrd", name=f"abrd{li}{hi}")
                hd[hi]["Bp"] = headp.tile([P, NJ], F32, tag="Bp", name=f"Bp{li}{hi}")
                hd[hi]["bp"] = headp.tile([P, NJ], F32, tag="bp", name=f"bp{li}{hi}")
                hd[hi]["hs2"] = [
                    headp.tile([P, 2, P], FP8, tag=f"hs2_{t}", name=f"hs2_{li}{t}_{hi}")
                    for t in range(NPAIR)]
            return hd

        def mk_prep(hd, XT, pars, phase):
            """Emit prep for slabs [2*phase, 2*phase+1] / chunks [8p, 8p+8)."""
            slabs = [2 * phase, 2 * phase + 1]
            chunks = range(8 * phase, 8 * phase + 8)

            def piece_a():
                # h = W^T x + b (PE + ACT bias-add from PSUM)
                for ib in slabs:
                    sl = slice(ib * IB, (ib + 1) * IB)
                    for hi, (Wc, b, Al, ar2) in enumerate(pars):
                        ps = ps_prep.tile([P, IB], F32, tag="prep", name="prep")
                        nc.tensor.matmul(ps[:], Wc[0][:], XT[0][:, sl], start=True, stop=False)
                        nc.tensor.matmul(ps[:], Wc[1][:], XT[1][:, sl], start=False, stop=True)
                        nc.scalar.activation(hd[hi]["hT"][:, sl], ps[:],
                                             AF.Identity, bias=b[:])

            def piece_b():
                # abrd = exp(-(1-alpha)*hl - C/2); B'/b' for these chunks
                for ib in slabs:
                    sl = slice(ib * IB, (ib + 1) * IB)
                    for hi, (Wc, b, Al, ar2) in enumerate(pars):
                        ps = ps_prep.tile([P, IB], F32, tag="prep", name="prep")
                        nc.tensor.matmul(ps[:], Al[:], hd[hi]["hT"][:, sl], start=True, stop=True)
                        nc.scalar.activation(hd[hi]["abrd"][:, sl], ps[:], AF.Exp,
                                             scale=-(1.0 - ALPHA), bias=negc2[:])
                for hi, (Wc, b, Al, ar2) in enumerate(pars):
                    psr = ps_prep.tile([P, IB], F32, tag="prep", name="prep")
                    for i, jc in enumerate(chunks):
                        nc.tensor.matmul(psr[:, 2 * i:2 * i + 2],
                                         hd[hi]["hT"][:, jc * P:(jc + 1) * P],
                                         ar2[:], start=True, stop=True)
                    pair = psr[:, 0:16].rearrange("p (c t) -> p c t", t=2)
                    csl = slice(8 * phase, 8 * phase + 8)
                    nc.scalar.activation(hd[hi]["Bp"][:, csl], pair[:, :, 0], AF.Exp,
                                         scale=1.0, bias=negc[:])
                    nc.scalar.activation(hd[hi]["bp"][:, csl], pair[:, :, 0], AF.Exp,
                                         scale=ALPHA, bias=negc2[:])

            def piece_c():
                # h chunks [node, d] via XBAR transpose -> fp8 pair slots
                for jc in chunks:
                    t, slot = jc // 2, jc % 2
                    for hi in range(2):
                        hsb = hsbp.tile([P, P], BF16, tag="hsb", name="hsb")
                        nc.sync.dma_start_transpose(
                            out=hsb[:], in_=hd[hi]["hT"][:, jc * P:(jc + 1) * P])
                        dst = hd[hi]["hs2"][t][:, slot, :]
                        if (2 * jc + hi) % 2 == 0:
                            nc.gpsimd.tensor_copy(dst, hsb[:])
                        else:
                            nc.scalar.activation(dst, hsb[:], AF.Copy)

            return [piece_a, piece_b, piece_c]

        def mk_epilogue(hd, OUTS, half, zp):
            oaccs = [hd[hi]["oacc"] for hi in range(2)]
            hTs = [hd[hi]["hT"] for hi in range(2)]

            def emit():
                for k in range(KH):
                    ib = half * KH + k
                    isl = slice(ib * IB, (ib + 1) * IB)
                    recip_f = smallp.tile([ZR, IB], F32, tag="recip_f",
                                          name="recip_f", bufs=2)
                    nc.vector.reciprocal_approx_fast(recip_f[:], zp[k][:])
                    recip = smallp.tile([ZR, IB], BF16, tag="recip",
                                        name="recip", bufs=2)
                    nc.scalar.activation(recip[:], recip_f[:], AF.Copy)
                    for hi in range(2):
                        rb = ps_prep.tile([P, IB], F32, tag="prep", name="prep")
                        nc.tensor.matmul(rb[:], sel4[hi][:], recip[:],
                                         start=True, stop=True)
                        rbs = epp.tile([P, IB], F32, tag="rbs", name="rbs")
                        nc.scalar.activation(rbs[:], rb[:], AF.Copy)
                        v = epp.tile([P, IB], BF16, tag="v", name="v")
                        nc.vector.tensor_tensor(v[:], oaccs[hi][k][:], rbs[:],
                                                AluOpType.mult)
                        v2 = epp.tile([P, IB], BF16, tag="v2", name="v2")
                        nc.gpsimd.tensor_tensor(v2[:], v[:], hTs[hi][:, isl],
                                                AluOpType.add)
                        # elu(x) = relu(x) + min(exp(x), 1) - 1
                        e = epp.tile([P, IB], BF16, tag="e", name="e")
                        nc.scalar.activation(e[:], v2[:], AF.Exp)
                        r3m1 = epp.tile([P, IB], BF16, tag="r3m1", name="r3m1")
                        nc.vector.tensor_scalar(r3m1[:], v2[:], 0.0, -1.0,
                                                AluOpType.max, AluOpType.add)
                        nc.vector.scalar_tensor_tensor(
                            OUTS[hi][:, isl], in0=e[:], scalar=1.0, in1=r3m1[:],
                            op0=AluOpType.min, op1=AluOpType.add)
            return emit

        PIPE_PAIRS = 3

        def gat_half(hd, OUTS, half, pending_epi, injections):
            """Emit one half's score loop. Returns this half's epilogue."""
            hsl = slice(half * HW, (half + 1) * HW)
            for hi in range(2):
                hd[hi]["oacc"] = [
                    ps_main.tile([P, IB], F32, tag=f"oacc{k}", name=f"oacc{k}_{hi}")
                    for k in range(KH)]
            zp = [ps_z.tile([ZR, IB], F32, tag=f"zp{k}", name=f"zp{k}")
                  for k in range(KH)]
            dlo, dhi = half * (NJ // NHALF), (half + 1) * (NJ // NHALF)
            deferred = []
            npipe = PIPE_PAIRS if pending_epi is not None else 0
            for t in range(NPAIR):
                jcs = [2 * t, 2 * t + 1]
                for hi in range(2):
                    H = hd[hi]
                    u8 = up.tile([P, 2, HW], FP8, tag="u8", name="u8")
                    for gi, jc in enumerate(jcs):
                        slot = jc % 2
                        dst = u8[:, slot, :]
                        path = PATHS[(2 * t + gi + 5 * hi) % 16]
                        b1 = H["bp"][:, jc:jc + 1]
                        b2 = H["Bp"][:, jc:jc + 1]
                        if path == "D":
                            nc.vector.tensor_scalar(
                                dst, H["abrd"][:, hsl], b1, b2,
                                AluOpType.mult, AluOpType.max)
                        else:
                            nc.gpsimd.tensor_scalar(
                                dst, H["abrd"][:, hsl], b1, b2,
                                AluOpType.mult, AluOpType.max)
                        if dlo <= jc < dhi:
                            off = jc * P - half * HW
                            nc.gpsimd.affine_select(
                                out=u8[:, slot, off:off + P],
                                in_=u8[:, slot, off:off + P],
                                compare_op=AluOpType.not_equal,
                                fill=0.0, base=0, pattern=[[-1, P]],
                                channel_multiplier=1)

                    def mms(H=H, u8=u8, t=t, hi=hi):
                        for k in range(KH):
                            nc.tensor.matmul(H["oacc"][k][:], H["hs2"][t][:],
                                             u8[:, :, k * IB:(k + 1) * IB],
                                             perf_mode=DR,
                                             start=(t == 0),
                                             stop=(t == NPAIR - 1))
                        for k in range(KH):
                            nc.tensor.matmul(zp[k][:], ones2[hi][:],
                                             u8[:, :, k * IB:(k + 1) * IB],
                                             perf_mode=DR,
                                             start=(t == 0 and hi == 0),
                                             stop=(t == NPAIR - 1 and hi == 1))
                    if t < npipe:
                        deferred.append(mms)
                    else:
                        if t == npipe and hi == 0:
                            if pending_epi is not None:
                                pending_epi()
                                pending_epi = None
                            for m in deferred:
                                m()
                            deferred = []
                            for inj in injections:
                                inj()
                            injections = []
                        mms()
            # no-injection halves with no pending epi: flush anything left
            for inj in injections:
                inj()
            return mk_epilogue(hd, OUTS, half, zp)

        pars0 = [(Wt[0, 0], bt[0, 0], Alt[0, 0], art[0, 0]),
                 (Wt[0, 1], bt[0, 1], Alt[0, 1], art[0, 1])]
        pars1 = [(Wt[1, 0], bt[1, 0], Alt[1, 0], art[1, 0]),
                 (Wt[1, 1], bt[1, 1], Alt[1, 1], art[1, 1])]

        hd0 = mk_state(0)
        prep0_e = mk_prep(hd0, xT, pars0, 0)
        prep0_l = mk_prep(hd0, xT, pars0, 1)
        for p in prep0_e + prep0_l:
            p()

        hd1 = mk_state(1)
        prep1_e = mk_prep(hd1, X1T, pars1, 0)
        prep1_l = mk_prep(hd1, X1T, pars1, 1)

        # ---- XBAR-transpose X2T back, convert to f32, store ----
        def store(c0, c1):
            for c in range(c0, c1):
                x2b = smallp.tile([P, F], BF16, tag="x2b", name="x2b")
                for f in range(2):
                    nc.sync.dma_start_transpose(
                        out=x2b[:, f * P:(f + 1) * P], in_=X2T[f][:, c * P:(c + 1) * P])
                ob = smallp.tile([P, F], F32, tag="ob", name="ob")
                if c % 2 == 0:
                    nc.vector.tensor_copy(ob[:], x2b[:])
                else:
                    nc.gpsimd.tensor_copy(ob[:], x2b[:])
                nc.sync.dma_start(out=out_d[c * P:(c + 1) * P, :], in_=ob[:])

        epi = gat_half(hd0, X1T, 0, None, [])
        epi = gat_half(hd0, X1T, 1, epi, prep1_e)
        epi = gat_half(hd1, X2T, 0, epi, prep1_l)
        # first output half (node columns 0:HW) is final after this epilogue
        epi = gat_half(hd1, X2T, 1, epi, [lambda: store(0, NJ // 2)])
        epi()
        store(NJ // 2, NJ)

    nc.compile()
    return nc


_CACHE = {}
LAST_RESULTS = None


def kernel(**inputs):
    global LAST_RESULTS
    from concourse.bass_utils import run_bass_kernel_spmd

    x = np.ascontiguousarray(np.asarray(inputs["x"], dtype=np.float32))
    B = x.shape[0]
    assert B == N_CORES and x.shape[1] == N and x.shape[2] == F

    if "nc" not in _CACHE:
        _CACHE["nc"] = build_nc()
    nc = _CACHE["nc"]

    base = {}
    for l in (0, 1):
        for h in (0, 1):
            base[f"W_{l}_{h}"] = np.ascontiguousarray(
                np.asarray(inputs[f"W_{l}_{h}"], dtype=np.float32))
            base[f"b_{l}_{h}"] = np.ascontiguousarray(
                np.asarray(inputs[f"b_{l}_{h}"], dtype=np.float32))
            base[f"a_{l}_{h}"] = np.ascontiguousarray(
                np.asarray(inputs[f"a_{l}_{h}"], dtype=np.float32))

    in_maps = [dict(base, x=np.ascontiguousarray(x[i])) for i in range(B)]
    res = run_bass_kernel_spmd(nc, in_maps, list(range(N_CORES)),
                               trace=bool(os.environ.get("BASS_TRACE")))
    LAST_RESULTS = res
    out = np.stack([res.results[i]["out"] for i in range(B)], axis=0)
    return out.astype(np.float32)
